# revision 2
# baseline (speedup 1.0000x reference)
"""Causal single-head self-attention on 8 TRN2 NeuronCores.

Sharding: 8 cores = 4 batches x 2 cores/batch. Within a batch the 8
512-query chunks are split zigzag (core A owns chunks {0,3,4,7}, core B
{1,2,5,6}) so causal work balances (18 units each). Each core projects
K/V for the whole batch from its own copy of x (recompute beats
cross-core K/V exchange at this size), computes Q only for its owned
chunks, then does block-causal flash-style attention without the
row-max pass (scores are O(1) here so exp never overflows) and a fused
out-projection.

SPMD trick: one program runs on all 8 cores, so per-core differences
live in the DATA only. x rows are fed in a per-core storage permutation
that puts each core's owned query chunks at uniform offsets
(storage chunks 0,2,4,6), and the causal mask for the 8 boundary
k-blocks of each slot is a per-core input tensor.

Layouts (partition dim first):
  xT   [128, 8, 4096]  bf16   x^T per d-chunk (from cast-DMA + xbar transpose)
  K^T  [128, 4096]     bf16   H-major keys
  Q^T  [128, 2048]     bf16   H-major owned queries
  V    [128, 32, 129]  bf16   token-major V tiles + ones column (rowsum trick)
  scores_T [k=128, q=512] PSUM; P_T = exp(scale*s) bf16; AV accumulates
  O [q=128, 128+1] in PSUM over k-blocks; col 128 = softmax denominator.
  Out-proj: lhsT = O^T tile (via xbar transpose), rhs = Wo^T; the
  1/denominator scale rides the PSUM->SBUF copy (tensor_scalar_mul).
"""

import numpy as np
from contextlib import ExitStack

import concourse.bass as bass
import concourse.tile as tile
from concourse import bacc, mybir
from concourse.bass_utils import run_bass_kernel_spmd
from concourse._compat import with_exitstack

S, B, D, H = 4096, 4, 1024, 128
P = 128
QC = 512                  # query chunk
NSLOT = 4                 # owned chunks per core
DC = D // P               # 8 d-chunks
TT = S // P               # 32 token tiles / k-blocks
SCALE = float(H) ** -0.5

# storage-order permutation of the 8 query chunks, per role. Queries the
# core owns sit at storage chunks 0,2,4,6; the first 2(g+1) storage
# chunks cover every true key needed by owned chunk g (extras masked).
SIGMA = {0: [0, 1, 3, 2, 4, 5, 7, 6], 1: [1, 0, 2, 3, 5, 4, 6, 7]}
QSLOT = [0, 2, 4, 6]      # storage chunk positions of owned queries

F32 = mybir.dt.float32
BF16 = mybir.dt.bfloat16


def _build_kernel():
    nc = bacc.Bacc("TRN2", target_bir_lowering=False, debug=False, num_devices=8)

    xb = nc.dram_tensor("xb", [S, D], F32, kind="ExternalInput")
    wqT = nc.dram_tensor("wqT", [D, H], F32, kind="ExternalInput")
    wkT = nc.dram_tensor("wkT", [D, H], F32, kind="ExternalInput")
    wvT = nc.dram_tensor("wvT", [D, H], F32, kind="ExternalInput")
    woT = nc.dram_tensor("woT", [H, D], F32, kind="ExternalInput")
    masks = nc.dram_tensor("masks", [NSLOT, 8, P, QC], BF16, kind="ExternalInput")
    out = nc.dram_tensor("out", [NSLOT * QC, D], F32, kind="ExternalOutput")

    with ExitStack() as ctx:
        tc = ctx.enter_context(tile.TileContext(nc))
        _body(ctx, tc, xb.ap(), wqT.ap(), wkT.ap(), wvT.ap(), woT.ap(),
              masks.ap(), out.ap())

    nc.compile()
    return nc


def _body(ctx, tc, xb, wqT, wkT, wvT, woT, masks, out):
    nc = tc.nc

    consts = ctx.enter_context(tc.tile_pool(name="consts", bufs=1))
    xstage = ctx.enter_context(tc.tile_pool(name="xstage", bufs=3))
    bigbuf = ctx.enter_context(tc.tile_pool(name="bigbuf", bufs=1))
    ptpool = ctx.enter_context(tc.tile_pool(name="pt", bufs=3))
    mpool = ctx.enter_context(tc.tile_pool(name="mask", bufs=3))
    otmp_pool = ctx.enter_context(tc.tile_pool(name="otmp", bufs=2))
    ypool = ctx.enter_context(tc.tile_pool(name="y", bufs=3))
    psA = ctx.enter_context(tc.tile_pool(name="psA", bufs=2, space="PSUM"))
    psO = ctx.enter_context(tc.tile_pool(name="psO", bufs=4, space="PSUM"))
    psV = ctx.enter_context(tc.tile_pool(name="psV", bufs=2, space="PSUM"))

    # ---- weights (cast f32 -> bf16 in the SWDGE DMA) ----
    wq_sb = consts.tile([P, DC, H], BF16)
    wk_sb = consts.tile([P, DC, H], BF16)
    wv_sb = consts.tile([P, DC, H], BF16)
    woT_sb = consts.tile([P, D], BF16)
    for w_sb, w_dram in ((wq_sb, wqT), (wk_sb, wkT), (wv_sb, wvT)):
        nc.gpsimd.dma_start(w_sb[:], w_dram.rearrange("(c p) h -> p c h", p=P))
    nc.gpsimd.dma_start(woT_sb[:], woT)

    # ---- x: cast-DMA then xbar-transpose into d-major ----
    xT = bigbuf.tile([P, DC, S], BF16)
    for t in range(TT):
        xt = xstage.tile([P, D], BF16)
        nc.gpsimd.dma_start(xt[:], xb[bass.ts(t, P), :])
        for c in range(DC):
            nc.sync.dma_start(
                xT[:, c, bass.ts(t, P)], xt[:, bass.ts(c, P)], transpose=True
            )

    # ---- projections ----
    k_sb = bigbuf.tile([P, S], BF16)
    q_sb = bigbuf.tile([P, NSLOT * QC], BF16)
    v_sb = bigbuf.tile([P, TT, H + 1], BF16)
    nc.vector.memset(v_sb[:, :, H], 1.0)  # ones column for rowsum trick

    for kt in range(S // QC):
        ps = psA.tile([P, QC], F32)
        for c in range(DC):
            nc.tensor.matmul(ps[:], lhsT=wk_sb[:, c, :], rhs=xT[:, c, bass.ts(kt, QC)],
                             start=(c == 0), stop=(c == DC - 1))
        nc.vector.tensor_copy(k_sb[:, bass.ts(kt, QC)], ps[:])

    for g in range(NSLOT):
        ps = psA.tile([P, QC], F32)
        for c in range(DC):
            nc.tensor.matmul(ps[:], lhsT=wq_sb[:, c, :],
                             rhs=xT[:, c, bass.ts(QSLOT[g], QC)],
                             start=(c == 0), stop=(c == DC - 1))
        nc.vector.tensor_copy(q_sb[:, bass.ts(g, QC)], ps[:])

    for t in range(TT):
        ps = psV.tile([P, H], F32)
        for c in range(DC):
            nc.tensor.matmul(ps[:], lhsT=xT[:, c, bass.ts(t, P)], rhs=wv_sb[:, c, :],
                             start=(c == 0), stop=(c == DC - 1))
        nc.vector.tensor_copy(v_sb[:, t, 0:H], ps[:])

    # ---- attention ----
    o_t = bigbuf.tile([P, NSLOT * QC], BF16)        # O^T, H-major, unnormalized
    rec_sb = bigbuf.tile([P, NSLOT * NSLOT], F32)   # 1/rowsum per q-tile column

    for g in range(NSLOT):
        nb = 8 * (g + 1)  # padded extent in k-blocks
        po = [psO.tile([P, H + 1], F32, name="po") for sub in range(NSLOT)]
        for bk in range(nb):
            ps = psA.tile([P, QC], F32)
            nc.tensor.matmul(ps[:], lhsT=k_sb[:, bass.ts(bk, P)],
                             rhs=q_sb[:, bass.ts(g, QC)], start=True, stop=True)
            pt = ptpool.tile([P, QC], BF16)
            nc.scalar.activation(pt[:], ps[:], mybir.ActivationFunctionType.Exp,
                                 scale=SCALE)
            if bk >= 8 * g:  # boundary region: causal mask from per-core data
                m = mpool.tile([P, QC], BF16)
                nc.sync.dma_start(m[:], masks[g, bk - 8 * g, :, :])
                nc.vector.tensor_mul(pt[:], pt[:], m[:])
            for sub in range(NSLOT):
                nc.tensor.matmul(po[sub][:], lhsT=pt[:, bass.ts(sub, P)],
                                 rhs=v_sb[:, bk, :],
                                 start=(bk == 0), stop=(bk == nb - 1))
        for sub in range(NSLOT):
            idx = g * NSLOT + sub
            nc.vector.reciprocal(rec_sb[:, idx : idx + 1], po[sub][:, H : H + 1])
            ot = otmp_pool.tile([P, H], BF16)
            nc.vector.tensor_copy(ot[:], po[sub][:, 0:H])
            nc.sync.dma_start(o_t[:, bass.ts(idx, P)], ot[:], transpose=True)

    # ---- output projection (normalization fused into the PSUM copy) ----
    for tt in range(NSLOT * QC // P):
        y = ypool.tile([P, D], F32)
        for half in range(2):
            ps = psA.tile([P, QC], F32)
            nc.tensor.matmul(ps[:], lhsT=o_t[:, bass.ts(tt, P)],
                             rhs=woT_sb[:, bass.ts(half, QC)], start=True, stop=True)
            nc.vector.tensor_scalar_mul(y[:, bass.ts(half, QC)], ps[:],
                                        rec_sb[:, tt : tt + 1])
        nc.sync.dma_start(out[bass.ts(tt, P), :], y[:])


_CACHED_NC = None


def _get_nc():
    global _CACHED_NC
    if _CACHED_NC is None:
        _CACHED_NC = _build_kernel()
    return _CACHED_NC


def _make_core_inputs(x, wqT, wkT, wvT, woT, core):
    import ml_dtypes

    b, role = core // 2, core % 2
    sigma = SIGMA[role]
    perm = np.concatenate([np.arange(QC) + c * QC for c in sigma])
    xb = np.ascontiguousarray(x[perm, b, :], dtype=np.float32)

    mask = np.zeros((NSLOT, 8, P, QC), np.float32)
    qq = np.arange(QC)[None, :]
    kk = np.arange(P)[:, None]
    for g in range(NSLOT):
        c_g = sigma[QSLOT[g]]
        for p in range(8):
            sc = sigma[2 * g + p // 4]
            k_true = sc * QC + (p % 4) * P + kk
            q_true = c_g * QC + qq
            mask[g, p] = (k_true <= q_true).astype(np.float32)
    return {
        "xb": xb,
        "wqT": wqT,
        "wkT": wkT,
        "wvT": wvT,
        "woT": woT,
        "masks": mask.astype(ml_dtypes.bfloat16),
    }


def kernel(x, Wq, Wk, Wv, Wo):
    x = np.asarray(x, dtype=np.float32)
    wqT = np.ascontiguousarray(np.asarray(Wq, np.float32).T)
    wkT = np.ascontiguousarray(np.asarray(Wk, np.float32).T)
    wvT = np.ascontiguousarray(np.asarray(Wv, np.float32).T)
    woT = np.ascontiguousarray(np.asarray(Wo, np.float32).T)

    nc = _get_nc()
    in_maps = [_make_core_inputs(x, wqT, wkT, wvT, woT, i) for i in range(8)]
    res = run_bass_kernel_spmd(nc, in_maps, list(range(8))).results

    out = np.empty((S, B, D), np.float32)
    for core in range(8):
        b, role = core // 2, core % 2
        sigma = SIGMA[role]
        co = res[core]["out"]
        for g in range(NSLOT):
            c_g = sigma[QSLOT[g]]
            out[c_g * QC : (c_g + 1) * QC, b, :] = co[g * QC : (g + 1) * QC, :]
    return out


# revision 7
# speedup vs baseline: 1.6050x; 1.6050x over previous
"""Causal single-head self-attention on 8 TRN2 NeuronCores.

Sharding: 8 cores = 4 batches x 2 cores/batch. Within a batch the 8
512-query chunks are split zigzag (core A owns chunks {0,3,4,7}, core B
{1,2,5,6}) so causal work balances (18 units each). Each core projects
K/V for the whole batch from its own copy of x (recompute beats
cross-core K/V exchange at this size), computes Q only for its owned
chunks, then does block-causal flash-style attention without the
row-max pass (scores are O(1) here so exp never overflows) and a fused
out-projection.

SPMD trick: one program runs on all 8 cores, so per-core differences
live in the DATA only. x rows are fed in a per-core storage permutation
that puts each core's owned query chunks at uniform offsets
(storage chunks 0,2,4,6), and the causal mask for the 8 boundary
k-blocks of each slot is a per-core input tensor.

Layouts (partition dim first):
  xT   [128, 8, 4096]  bf16   x^T per d-chunk (from cast-DMA + xbar transpose)
  K^T  [128, 4096]     bf16   H-major keys
  Q^T  [128, 2048]     bf16   H-major owned queries
  V    [128, 32, 129]  bf16   token-major V tiles + ones column (rowsum trick)
  scores_T [k=128, q=512] PSUM; P_T = exp(scale*s) bf16; AV accumulates
  O [q=128, 128+1] in PSUM over k-blocks; col 128 = softmax denominator.
  Out-proj: lhsT = O^T tile (via xbar transpose), rhs = Wo^T; the
  1/denominator scale rides the PSUM->SBUF copy (tensor_scalar_mul).
"""

import numpy as np
from contextlib import ExitStack

import concourse.bass as bass
import concourse.tile as tile
from concourse import bacc, mybir
from concourse.bass_utils import run_bass_kernel_spmd
from concourse._compat import with_exitstack

S, B, D, H = 4096, 4, 1024, 128
P = 128
QC = 512                  # query chunk
NSLOT = 4                 # owned chunks per core
DC = D // P               # 8 d-chunks
TT = S // P               # 32 token tiles / k-blocks
SCALE = float(H) ** -0.5

# storage-order permutation of the 8 query chunks, per role. Queries the
# core owns sit at storage chunks 0,2,4,6; the first 2(g+1) storage
# chunks cover every true key needed by owned chunk g (extras masked).
SIGMA = {0: [0, 1, 3, 2, 4, 5, 7, 6], 1: [1, 0, 2, 3, 5, 4, 6, 7]}
QSLOT = [0, 2, 4, 6]      # storage chunk positions of owned queries

F32 = mybir.dt.float32
BF16 = mybir.dt.bfloat16


def _build_kernel():
    nc = bacc.Bacc("TRN2", target_bir_lowering=False, debug=False, num_devices=8)

    xb = nc.dram_tensor("xb", [S, D], F32, kind="ExternalInput")
    wqT = nc.dram_tensor("wqT", [D, H], F32, kind="ExternalInput")
    wkT = nc.dram_tensor("wkT", [D, H], F32, kind="ExternalInput")
    wvT = nc.dram_tensor("wvT", [D, H], F32, kind="ExternalInput")
    woT = nc.dram_tensor("woT", [H, D], F32, kind="ExternalInput")
    masks = nc.dram_tensor("masks", [NSLOT, 8, P, QC], BF16, kind="ExternalInput")
    out = nc.dram_tensor("out", [NSLOT * QC, D], F32, kind="ExternalOutput")

    with ExitStack() as ctx:
        tc = ctx.enter_context(tile.TileContext(nc))
        _body(ctx, tc, xb.ap(), wqT.ap(), wkT.ap(), wvT.ap(), woT.ap(),
              masks.ap(), out.ap())

    nc.compile()
    return nc


def _body(ctx, tc, xb, wqT, wkT, wvT, woT, masks, out):
    nc = tc.nc

    consts = ctx.enter_context(tc.tile_pool(name="consts", bufs=1))
    xstage = ctx.enter_context(tc.tile_pool(name="xstage", bufs=4))
    bigbuf = ctx.enter_context(tc.tile_pool(name="bigbuf", bufs=1))
    ptpool = ctx.enter_context(tc.tile_pool(name="pt", bufs=4))
    mpool = ctx.enter_context(tc.tile_pool(name="mask", bufs=3))
    otmp_pool = ctx.enter_context(tc.tile_pool(name="otmp", bufs=2))
    ypool = ctx.enter_context(tc.tile_pool(name="y", bufs=3))
    psA = ctx.enter_context(tc.tile_pool(name="psA", bufs=3, space="PSUM"))
    psO = ctx.enter_context(tc.tile_pool(name="psO", bufs=4, space="PSUM"))

    # ---- weights (cast f32 -> bf16 in the SWDGE DMA) ----
    wq_sb = consts.tile([P, DC, H], BF16)
    wk_sb = consts.tile([P, DC, H], BF16)
    wv_sb = consts.tile([P, DC, H], BF16)
    woT_sb = consts.tile([P, D], BF16)
    for w_sb, w_dram in ((wq_sb, wqT), (wk_sb, wkT), (wv_sb, wvT)):
        nc.gpsimd.dma_start(w_sb[:], w_dram.rearrange("(c p) h -> p c h", p=P))
    nc.gpsimd.dma_start(woT_sb[:], woT)

    xT = bigbuf.tile([P, DC, S], BF16)
    k_sb = bigbuf.tile([P, S], BF16)
    vT_sb = bigbuf.tile([P, S], BF16)
    q_sb = bigbuf.tile([P, NSLOT * QC], BF16)
    v_sb = bigbuf.tile([P, TT, 2 * P], BF16)  # block stride 512B (xbar alignment)
    o_t = bigbuf.tile([P, NSLOT * NSLOT, P], BF16)  # O^T [h, q-tile, q], unnormalized
    rec_sb = bigbuf.tile([P, NSLOT * NSLOT], F32)   # 1/rowsum per q-tile column
    nc.vector.memset(v_sb[:, :, H], 1.0)  # ones column for rowsum trick

    def load_x_tile(t):
        """cast-DMA x token-tile then one batched xbar-transpose into xT"""
        xt = xstage.tile([P, D], BF16)
        nc.gpsimd.dma_start(xt[:], xb[bass.ts(t, P), :])
        nc.sync.dma_start(xT[:, :, bass.ts(t, P)], xt[:], transpose=True)

    def project(w_sb, dst, kt):
        ps = psA.tile([P, QC], F32)
        for c in range(DC):
            nc.tensor.matmul(ps[:], lhsT=w_sb[:, c, :], rhs=xT[:, c, bass.ts(kt, QC)],
                             start=(c == 0), stop=(c == DC - 1))
        nc.vector.tensor_copy(dst[:, bass.ts(kt, QC)], ps[:])

    # Pipelined emission: per slot g, load the two new x 512-chunks, project
    # their K/V (+V re-transpose to token-major), project this slot's Q, run
    # the slot's attention, then its out-projection. Keeps PE dense and lets
    # attention start as soon as the first chunks land.
    for g in range(NSLOT):
        for t in range(8 * g, 8 * g + 8):
            load_x_tile(t)
        for kt in (2 * g, 2 * g + 1):
            project(wk_sb, k_sb, kt)
            project(wv_sb, vT_sb, kt)
            nc.sync.dma_start(v_sb[:, 4 * kt : 4 * kt + 4, 0:H],
                              vT_sb[:, bass.ts(kt, QC)], transpose=True)
        ps = psA.tile([P, QC], F32)
        for c in range(DC):
            nc.tensor.matmul(ps[:], lhsT=wq_sb[:, c, :],
                             rhs=xT[:, c, bass.ts(QSLOT[g], QC)],
                             start=(c == 0), stop=(c == DC - 1))
        nc.vector.tensor_copy(q_sb[:, bass.ts(g, QC)], ps[:])

        # ---- attention for slot g ----
        nb = 8 * (g + 1)  # padded extent in k-blocks
        po = [psO.tile([P, H + 1], F32, name="po") for sub in range(NSLOT)]
        for bk in range(nb):
            ps = psA.tile([P, QC], F32)
            nc.tensor.matmul(ps[:], lhsT=k_sb[:, bass.ts(bk, P)],
                             rhs=q_sb[:, bass.ts(g, QC)], start=True, stop=True)
            pt = ptpool.tile([P, QC], BF16)
            nc.scalar.activation(pt[:], ps[:], mybir.ActivationFunctionType.Exp,
                                 scale=SCALE)
            if bk >= 8 * g:  # boundary region: causal mask from per-core data
                m = mpool.tile([P, QC], BF16)
                nc.gpsimd.dma_start(m[:], masks[g, bk - 8 * g, :, :])
                nc.vector.tensor_mul(pt[:], pt[:], m[:])
            for sub in range(NSLOT):
                nc.tensor.matmul(po[sub][:], lhsT=pt[:, bass.ts(sub, P)],
                                 rhs=v_sb[:, bk, 0 : H + 1],
                                 start=(bk == 0), stop=(bk == nb - 1))
        ot = otmp_pool.tile([P, NSLOT * P], BF16)
        for sub in range(NSLOT):
            idx = g * NSLOT + sub
            nc.vector.reciprocal(rec_sb[:, idx : idx + 1], po[sub][:, H : H + 1])
            nc.vector.tensor_copy(ot[:, bass.ts(sub, P)], po[sub][:, 0:H])
        nc.sync.dma_start(o_t[:, g * NSLOT : (g + 1) * NSLOT, :], ot[:],
                          transpose=True)

        # ---- out-projection for slot g (normalization fused in PSUM copy) ----
        for tt in range(g * NSLOT, (g + 1) * NSLOT):
            y = ypool.tile([P, D], F32)
            for half in range(2):
                ps = psA.tile([P, QC], F32)
                nc.tensor.matmul(ps[:], lhsT=o_t[:, tt, :],
                                 rhs=woT_sb[:, bass.ts(half, QC)],
                                 start=True, stop=True)
                nc.vector.tensor_scalar_mul(y[:, bass.ts(half, QC)], ps[:],
                                            rec_sb[:, tt : tt + 1])
            nc.sync.dma_start(out[bass.ts(tt, P), :], y[:])


_CACHED_NC = None


def _get_nc():
    global _CACHED_NC
    if _CACHED_NC is None:
        _CACHED_NC = _build_kernel()
    return _CACHED_NC


def _make_core_inputs(x, wqT, wkT, wvT, woT, core):
    import ml_dtypes

    b, role = core // 2, core % 2
    sigma = SIGMA[role]
    perm = np.concatenate([np.arange(QC) + c * QC for c in sigma])
    xb = np.ascontiguousarray(x[perm, b, :], dtype=np.float32)

    mask = np.zeros((NSLOT, 8, P, QC), np.float32)
    qq = np.arange(QC)[None, :]
    kk = np.arange(P)[:, None]
    for g in range(NSLOT):
        c_g = sigma[QSLOT[g]]
        for p in range(8):
            sc = sigma[2 * g + p // 4]
            k_true = sc * QC + (p % 4) * P + kk
            q_true = c_g * QC + qq
            mask[g, p] = (k_true <= q_true).astype(np.float32)
    return {
        "xb": xb,
        "wqT": wqT,
        "wkT": wkT,
        "wvT": wvT,
        "woT": woT,
        "masks": mask.astype(ml_dtypes.bfloat16),
    }


def kernel(x, Wq, Wk, Wv, Wo):
    x = np.asarray(x, dtype=np.float32)
    wqT = np.ascontiguousarray(np.asarray(Wq, np.float32).T)
    wkT = np.ascontiguousarray(np.asarray(Wk, np.float32).T)
    wvT = np.ascontiguousarray(np.asarray(Wv, np.float32).T)
    woT = np.ascontiguousarray(np.asarray(Wo, np.float32).T)

    nc = _get_nc()
    in_maps = [_make_core_inputs(x, wqT, wkT, wvT, woT, i) for i in range(8)]
    res = run_bass_kernel_spmd(nc, in_maps, list(range(8))).results

    out = np.empty((S, B, D), np.float32)
    for core in range(8):
        b, role = core // 2, core % 2
        sigma = SIGMA[role]
        co = res[core]["out"]
        for g in range(NSLOT):
            c_g = sigma[QSLOT[g]]
            out[c_g * QC : (c_g + 1) * QC, b, :] = co[g * QC : (g + 1) * QC, :]
    return out


# revision 12
# speedup vs baseline: 3.1298x; 1.9500x over previous
"""Causal single-head self-attention on 8 TRN2 NeuronCores.

Sharding: 8 cores = 4 batches x 2 cores/batch. Within a batch the 8
512-query chunks are split zigzag (core A owns chunks {0,3,4,7}, core B
{1,2,5,6}) so causal work balances (18 units each). Each core projects
K/V for the whole batch from its own copy of x (recompute beats
cross-core K/V exchange at this size), computes Q only for its owned
chunks, then does block-causal flash-style attention without the
row-max pass (scores here are O(1) so exp never overflows) and a fused
out-projection.

SPMD trick: one program runs on all 8 cores, so per-core differences
live in the DATA only. x rows are fed in a per-core storage permutation
that puts each core's owned query chunks at uniform offsets (storage
chunks 0,2,4,6), and the causal mask for the 8 boundary k-blocks of
each slot is applied with tensor_mask driven by a tiny per-core
threshold tensor. x is passed D-major (transposed on host during
sharding) so no on-chip transposes are needed for it — on-chip xbar
transposes alternate the DMA crossbar mode with plain copies, which
serializes the whole DMA subsystem.

Layouts (partition dim first):
  xT   [128, 8, 4096]  bf16   x^T per d-chunk (cast f32->bf16 in SWDGE DMA)
  K^T  [128, 4096]     bf16   H-major keys
  Q^T  [128, 2048]     bf16   H-major owned queries
  V    [128, 32, 256]  bf16   token-major V tiles; col 128 = ones (rowsum
                              trick), block stride padded for xbar alignment
  scores_T [k=128, q=512] PSUM; P_T = exp(scale*s) bf16 (ACT, fused scale)
  O [q=128, 128+1] accumulates in PSUM over k-blocks; col 128 = denominator.
  Out-proj: lhsT = O^T tile (xbar transpose), rhs = Wo^T; the 1/denominator
  scale rides the PSUM->SBUF copy (tensor_scalar_mul).
"""

import numpy as np
from contextlib import ExitStack

import concourse.bass as bass
import concourse.tile as tile
from concourse import bacc, mybir
from concourse.bass_utils import run_bass_kernel_spmd

S, B, D, H = 4096, 4, 1024, 128
P = 128
QC = 512                  # query chunk
NSLOT = 4                 # owned chunks per core
DC = D // P               # 8 d-chunks
TT = S // P               # 32 token tiles / k-blocks
NKT = S // QC             # 8 key 512-chunks
SCALE = float(H) ** -0.5

# storage-order permutation of the 8 query chunks, per role. Queries the
# core owns sit at storage chunks 0,2,4,6; the first 2(g+1) storage
# chunks cover every true key needed by owned chunk g (extras masked).
SIGMA = {0: [0, 1, 3, 2, 4, 5, 7, 6], 1: [1, 0, 2, 3, 5, 4, 6, 7]}
QSLOT = [0, 2, 4, 6]      # storage chunk positions of owned queries

F32 = mybir.dt.float32
BF16 = mybir.dt.bfloat16


def _build_kernel():
    nc = bacc.Bacc("TRN2", target_bir_lowering=False, debug=False, num_devices=8)

    xbT = nc.dram_tensor("xbT", [D, S], F32, kind="ExternalInput")
    wqT = nc.dram_tensor("wqT", [D, H], F32, kind="ExternalInput")
    wkT = nc.dram_tensor("wkT", [D, H], F32, kind="ExternalInput")
    wvT = nc.dram_tensor("wvT", [D, H], F32, kind="ExternalInput")
    woT = nc.dram_tensor("woT", [H, D], F32, kind="ExternalInput")
    mlen = nc.dram_tensor("mlen", [P, NSLOT * 8], F32, kind="ExternalInput")
    out = nc.dram_tensor("out", [NSLOT * QC, D], F32, kind="ExternalOutput")

    with ExitStack() as ctx:
        tc = ctx.enter_context(tile.TileContext(nc))
        _body(ctx, tc, xbT.ap(), wqT.ap(), wkT.ap(), wvT.ap(), woT.ap(),
              mlen.ap(), out.ap())

    nc.compile()
    return nc


def _body(ctx, tc, xbT, wqT, wkT, wvT, woT, mlen, out):
    nc = tc.nc

    consts = ctx.enter_context(tc.tile_pool(name="consts", bufs=1))
    bigbuf = ctx.enter_context(tc.tile_pool(name="bigbuf", bufs=1))
    ptpool = ctx.enter_context(tc.tile_pool(name="pt", bufs=4))
    otmp_pool = ctx.enter_context(tc.tile_pool(name="otmp", bufs=2))
    ypool = ctx.enter_context(tc.tile_pool(name="y", bufs=3))
    psA = ctx.enter_context(tc.tile_pool(name="psA", bufs=4, space="PSUM"))
    psO = ctx.enter_context(tc.tile_pool(name="psO", bufs=4, space="PSUM"))

    # ---- constants (cast f32 -> bf16 in the SWDGE DMA) ----
    wq_sb = consts.tile([P, DC, H], BF16)
    wk_sb = consts.tile([P, DC, H], BF16)
    wv_sb = consts.tile([P, DC, H], BF16)
    woT_sb = consts.tile([P, D], BF16)
    for w_sb, w_dram in ((wq_sb, wqT), (wk_sb, wkT), (wv_sb, wvT)):
        nc.gpsimd.dma_start(w_sb[:], w_dram.rearrange("(c p) h -> p c h", p=P))
    nc.gpsimd.dma_start(woT_sb[:], woT)
    mlen_sb = consts.tile([P, NSLOT * 8], F32)
    nc.gpsimd.dma_start(mlen_sb[:], mlen)
    qneg = consts.tile([P, QC], F32)  # 0,-1,...,-511 along free, all partitions
    nc.gpsimd.iota(qneg[:], pattern=[[-1, QC]], base=0, channel_multiplier=0,
                   allow_small_or_imprecise_dtypes=True)
    # Build the 32 boundary-block causal masks on-chip: keep (1.0) where
    # -q_local - mlen < 0  <=>  q_true >= k_true. (tensor_mask ISA op is
    # rejected by this HW path, so build masks once and tensor_mul per block.)
    mask_sb = consts.tile([P, NSLOT * 8, QC], BF16)
    for idx in range(NSLOT * 8):
        nc.vector.tensor_scalar(mask_sb[:, idx, :], qneg[:],
                                mlen_sb[:, idx : idx + 1], 0.0,
                                op0=mybir.AluOpType.subtract,
                                op1=mybir.AluOpType.is_lt)

    xT = bigbuf.tile([P, DC, S], BF16)
    k_sb = bigbuf.tile([P, S], BF16)
    vT_sb = bigbuf.tile([P, S], BF16)
    q_sb = bigbuf.tile([P, NSLOT * QC], BF16)
    v_sb = bigbuf.tile([P, TT, 2 * P], BF16)  # block stride 512B (xbar alignment)
    o_t = bigbuf.tile([P, NSLOT * NSLOT, P], BF16)  # O^T [h, q-tile, q], unnorm
    rec_sb = bigbuf.tile([P, NSLOT * NSLOT], F32)   # 1/rowsum per q-tile column
    nc.vector.memset(v_sb[:, :, H], 1.0)  # ones column for rowsum trick

    def project(w_sb, dst, src_kt, dst_kt=None):
        ps = psA.tile([P, QC], F32)
        for c in range(DC):
            nc.tensor.matmul(ps[:], lhsT=w_sb[:, c, :],
                             rhs=xT[:, c, bass.ts(src_kt, QC)],
                             start=(c == 0), stop=(c == DC - 1))
        nc.vector.tensor_copy(dst[:, bass.ts(src_kt if dst_kt is None else dst_kt,
                                             QC)], ps[:])

    def attention_slot(g):
        nb = 8 * (g + 1)  # padded extent in k-blocks
        po = [psO.tile([P, H + 1], F32, name="po") for _ in range(NSLOT)]
        for bk in range(nb):
            ps = psA.tile([P, QC], F32)
            nc.tensor.matmul(ps[:], lhsT=k_sb[:, bass.ts(bk, P)],
                             rhs=q_sb[:, bass.ts(g, QC)], start=True, stop=True)
            pt = ptpool.tile([P, QC], BF16)
            nc.scalar.activation(pt[:], ps[:], mybir.ActivationFunctionType.Exp,
                                 scale=SCALE)
            if bk >= 8 * g:  # boundary: causal mask via per-core mask tiles
                idx = g * 8 + (bk - 8 * g)
                nc.vector.tensor_mul(pt[:], pt[:], mask_sb[:, idx, :])
            for sub in range(NSLOT):
                nc.tensor.matmul(po[sub][:], lhsT=pt[:, bass.ts(sub, P)],
                                 rhs=v_sb[:, bk, 0 : H + 1],
                                 start=(bk == 0), stop=(bk == nb - 1))
        ot = otmp_pool.tile([P, NSLOT * P], BF16)
        for sub in range(NSLOT):
            idx = g * NSLOT + sub
            nc.vector.reciprocal(rec_sb[:, idx : idx + 1], po[sub][:, H : H + 1])
            nc.vector.tensor_copy(ot[:, bass.ts(sub, P)], po[sub][:, 0:H])
        nc.sync.dma_start(o_t[:, g * NSLOT : (g + 1) * NSLOT, :], ot[:],
                          transpose=True)

    def outproj_slot(g):
        for tt in range(g * NSLOT, (g + 1) * NSLOT):
            y = ypool.tile([P, D], F32)
            for half in range(2):
                ps = psA.tile([P, QC], F32)
                nc.tensor.matmul(ps[:], lhsT=o_t[:, tt, :],
                                 rhs=woT_sb[:, bass.ts(half, QC)],
                                 start=True, stop=True)
                nc.vector.tensor_scalar_mul(y[:, bass.ts(half, QC)], ps[:],
                                            rec_sb[:, tt : tt + 1])
            nc.sync.dma_start(out[bass.ts(tt, P), :], y[:])

    # Pipelined emission over key 512-chunks: load the chunk's x columns
    # (already D-major; cast-DMA only), project K/V (+V re-transpose to
    # token-major), project Q when its chunk lands, then run each slot's
    # attention + out-projection as soon as its extent is covered.
    for kt in range(NKT):
        for c in range(DC):
            nc.gpsimd.dma_start(xT[:, c, bass.ts(kt, QC)],
                                xbT[bass.ts(c, P), bass.ts(kt, QC)])
        project(wk_sb, k_sb, kt)
        project(wv_sb, vT_sb, kt)
        nc.sync.dma_start(v_sb[:, 4 * kt : 4 * kt + 4, 0:H],
                          vT_sb[:, bass.ts(kt, QC)], transpose=True)
        if kt % 2 == 0:
            project(wq_sb, q_sb, kt, dst_kt=kt // 2)  # QSLOT[g] == 2g == kt
        else:
            g = (kt - 1) // 2
            attention_slot(g)
            outproj_slot(g)


_CACHED_NC = None


def _get_nc():
    global _CACHED_NC
    if _CACHED_NC is None:
        _CACHED_NC = _build_kernel()
    return _CACHED_NC


def _make_core_inputs(x, wqT, wkT, wvT, woT, core):
    b, role = core // 2, core % 2
    sigma = SIGMA[role]
    perm = np.concatenate([np.arange(QC) + c * QC for c in sigma])
    xbT = np.ascontiguousarray(x[perm, b, :].T, dtype=np.float32)

    # tensor_mask thresholds: keep q_local where -q_local < mlen[k_local],
    # i.e. q_true >= k_true  =>  mlen = Q0 - k_true + 1
    mlen = np.zeros((P, NSLOT * 8), np.float32)
    kk = np.arange(P)
    for g in range(NSLOT):
        q0 = sigma[QSLOT[g]] * QC
        for p in range(8):
            sc = sigma[2 * g + p // 4]
            k_true = sc * QC + (p % 4) * P + kk
            mlen[:, g * 8 + p] = q0 - k_true + 1
    return {"xbT": xbT, "wqT": wqT, "wkT": wkT, "wvT": wvT, "woT": woT,
            "mlen": mlen}


def kernel(x, Wq, Wk, Wv, Wo):
    x = np.asarray(x, dtype=np.float32)
    wqT = np.ascontiguousarray(np.asarray(Wq, np.float32).T)
    wkT = np.ascontiguousarray(np.asarray(Wk, np.float32).T)
    wvT = np.ascontiguousarray(np.asarray(Wv, np.float32).T)
    woT = np.ascontiguousarray(np.asarray(Wo, np.float32).T)

    nc = _get_nc()
    in_maps = [_make_core_inputs(x, wqT, wkT, wvT, woT, i) for i in range(8)]
    res = run_bass_kernel_spmd(nc, in_maps, list(range(8))).results

    out = np.empty((S, B, D), np.float32)
    for core in range(8):
        b, role = core // 2, core % 2
        sigma = SIGMA[role]
        co = res[core]["out"]
        for g in range(NSLOT):
            c_g = sigma[QSLOT[g]]
            out[c_g * QC : (c_g + 1) * QC, b, :] = co[g * QC : (g + 1) * QC, :]
    return out


# revision 18
# speedup vs baseline: 3.2914x; 1.0516x over previous
"""Causal single-head self-attention on 8 TRN2 NeuronCores.

Sharding: 8 cores = 4 batches x 2 cores/batch. Within a batch the 8
512-query chunks are split zigzag (core A owns chunks {0,3,4,7}, core B
{1,2,5,6}) so causal work balances (18 units each). Each core projects
K/V for the whole batch from its own copy of x (recompute beats
cross-core K/V exchange at this size), computes Q only for its owned
chunks, then does block-causal flash-style attention without the
row-max pass (scores here are O(1) so exp never overflows) and a fused
out-projection.

SPMD trick: one program runs on all 8 cores, so per-core differences
live in the DATA only. x rows are fed in a per-core storage permutation
that puts each core's owned query chunks at uniform offsets (storage
chunks 0,2,4,6), and the causal mask for the 8 boundary k-blocks of
each slot is applied with tensor_mask driven by a tiny per-core
threshold tensor. x is passed D-major (transposed on host during
sharding) so no on-chip transposes are needed for it — on-chip xbar
transposes alternate the DMA crossbar mode with plain copies, which
serializes the whole DMA subsystem.

Layouts (partition dim first):
  xT   [128, 8, 4096]  bf16   x^T per d-chunk (cast f32->bf16 in SWDGE DMA)
  K^T  [128, 4096]     bf16   H-major keys
  Q^T  [128, 2048]     bf16   H-major owned queries
  V    [128, 32, 256]  bf16   token-major V tiles; col 128 = ones (rowsum
                              trick), block stride padded for xbar alignment
  scores_T [k=128, q=512] PSUM; P_T = exp(scale*s) bf16 (ACT, fused scale)
  O [q=128, 128+1] accumulates in PSUM over k-blocks; col 128 = denominator.
  Out-proj: lhsT = O^T tile (xbar transpose), rhs = Wo^T; the 1/denominator
  scale rides the PSUM->SBUF copy (tensor_scalar_mul).
"""

import numpy as np
from contextlib import ExitStack

import concourse.bass as bass
import concourse.tile as tile
from concourse import bacc, mybir
from concourse.bass_utils import run_bass_kernel_spmd
from concourse.masks import make_identity

S, B, D, H = 4096, 4, 1024, 128
P = 128
QC = 512                  # query chunk
NSLOT = 4                 # owned chunks per core
DC = D // P               # 8 d-chunks
TT = S // P               # 32 token tiles / k-blocks
NKT = S // QC             # 8 key 512-chunks
SCALE = float(H) ** -0.5

# storage-order permutation of the 8 query chunks, per role. Queries the
# core owns sit at storage chunks 0,2,4,6; the first 2(g+1) storage
# chunks cover every true key needed by owned chunk g (extras masked).
SIGMA = {0: [0, 1, 3, 2, 4, 5, 7, 6], 1: [1, 0, 2, 3, 5, 4, 6, 7]}
QSLOT = [0, 2, 4, 6]      # storage chunk positions of owned queries

F32 = mybir.dt.float32
BF16 = mybir.dt.bfloat16


def _build_kernel():
    nc = bacc.Bacc("TRN2", target_bir_lowering=False, debug=False, num_devices=8)

    xbT = nc.dram_tensor("xbT", [D, S], F32, kind="ExternalInput")
    wqT = nc.dram_tensor("wqT", [D, H], F32, kind="ExternalInput")
    wkT = nc.dram_tensor("wkT", [D, H], F32, kind="ExternalInput")
    wvT = nc.dram_tensor("wvT", [D, H], F32, kind="ExternalInput")
    woT = nc.dram_tensor("woT", [H, D], F32, kind="ExternalInput")
    mlen = nc.dram_tensor("mlen", [P, NSLOT * 8], F32, kind="ExternalInput")
    out = nc.dram_tensor("out", [NSLOT * QC, D], F32, kind="ExternalOutput")

    with ExitStack() as ctx:
        tc = ctx.enter_context(tile.TileContext(nc))
        _body(ctx, tc, xbT.ap(), wqT.ap(), wkT.ap(), wvT.ap(), woT.ap(),
              mlen.ap(), out.ap())

    nc.compile()
    return nc


def _body(ctx, tc, xbT, wqT, wkT, wvT, woT, mlen, out):
    nc = tc.nc

    consts = ctx.enter_context(tc.tile_pool(name="consts", bufs=1))
    bigbuf = ctx.enter_context(tc.tile_pool(name="bigbuf", bufs=1))
    ptpool = ctx.enter_context(tc.tile_pool(name="pt", bufs=4))
    otmp_pool = ctx.enter_context(tc.tile_pool(name="otmp", bufs=2))
    ypool = ctx.enter_context(tc.tile_pool(name="y", bufs=3))
    psA = ctx.enter_context(tc.tile_pool(name="psA", bufs=2, space="PSUM"))
    psO = ctx.enter_context(tc.tile_pool(name="psO", bufs=4, space="PSUM"))

    # ---- constants (cast f32 -> bf16 in the SWDGE DMA) ----
    wq_sb = consts.tile([P, DC, H], BF16)
    wk_sb = consts.tile([P, DC, H], BF16)
    wv_sb = consts.tile([P, DC, H], BF16)
    woT_sb = consts.tile([P, D], BF16)
    for w_sb, w_dram in ((wq_sb, wqT), (wk_sb, wkT), (wv_sb, wvT)):
        nc.gpsimd.dma_start(w_sb[:], w_dram.rearrange("(c p) h -> p c h", p=P))
    nc.gpsimd.dma_start(woT_sb[:], woT)
    mlen_sb = consts.tile([P, NSLOT * 8], F32)
    nc.gpsimd.dma_start(mlen_sb[:], mlen)
    qneg = consts.tile([P, QC], F32)  # 0,-1,...,-511 along free, all partitions
    nc.gpsimd.iota(qneg[:], pattern=[[-1, QC]], base=0, channel_multiplier=0,
                   allow_small_or_imprecise_dtypes=True)
    # Build the 32 boundary-block causal masks on-chip: keep (1.0) where
    # -q_local - mlen < 0  <=>  q_true >= k_true. (tensor_mask ISA op is
    # rejected by this HW path, so build masks once and tensor_mul per block.)
    mask_sb = consts.tile([P, NSLOT * 8, QC], BF16)
    for idx in range(NSLOT * 8):
        nc.vector.tensor_scalar(mask_sb[:, idx, :], qneg[:],
                                mlen_sb[:, idx : idx + 1], 0.0,
                                op0=mybir.AluOpType.subtract,
                                op1=mybir.AluOpType.is_lt)
    ident = consts.tile([P, P], BF16)
    make_identity(nc, ident[:])

    xT = bigbuf.tile([P, DC, S], BF16)
    k_sb = bigbuf.tile([P, S], BF16)
    vT_sb = bigbuf.tile([P, S], BF16)
    q_sb = bigbuf.tile([P, NSLOT * QC], BF16)
    v_sb = bigbuf.tile([P, TT, 2 * P], BF16)  # block stride 512B (xbar alignment)
    o_t = bigbuf.tile([P, NSLOT * NSLOT, P], BF16)  # O^T [h, q-tile, q], unnorm
    rec_sb = bigbuf.tile([P, NSLOT * NSLOT], F32)   # 1/rowsum per q-tile column
    nc.vector.memset(v_sb[:, :, H], 1.0)  # ones column for rowsum trick

    def project(w_sb, dst, src_kt, dst_kt=None):
        ps = psA.tile([P, QC], F32)
        for c in range(DC):
            nc.tensor.matmul(ps[:], lhsT=w_sb[:, c, :],
                             rhs=xT[:, c, bass.ts(src_kt, QC)],
                             start=(c == 0), stop=(c == DC - 1))
        nc.vector.tensor_copy(dst[:, bass.ts(src_kt if dst_kt is None else dst_kt,
                                             QC)], ps[:])

    def attention_slot(g):
        nb = 8 * (g + 1)  # padded extent in k-blocks
        po = [psO.tile([P, H + 1], F32, name="po") for _ in range(NSLOT)]
        for bk in range(nb):
            ps = psA.tile([P, QC], F32)
            nc.tensor.matmul(ps[:], lhsT=k_sb[:, bass.ts(bk, P)],
                             rhs=q_sb[:, bass.ts(g, QC)], start=True, stop=True)
            pt = ptpool.tile([P, QC], BF16)
            nc.scalar.activation(pt[:], ps[:], mybir.ActivationFunctionType.Exp,
                                 scale=SCALE)
            if bk >= 8 * g:  # boundary: causal mask via per-core mask tiles
                idx = g * 8 + (bk - 8 * g)
                nc.vector.tensor_mul(pt[:], pt[:], mask_sb[:, idx, :])
            for sub in range(NSLOT):
                nc.tensor.matmul(po[sub][:], lhsT=pt[:, bass.ts(sub, P)],
                                 rhs=v_sb[:, bk, 0 : H + 1],
                                 start=(bk == 0), stop=(bk == nb - 1))
        for sub in range(NSLOT):
            idx = g * NSLOT + sub
            nc.vector.reciprocal(rec_sb[:, idx : idx + 1], po[sub][:, H : H + 1])
            ob = otmp_pool.tile([P, P], BF16, name="ob")
            nc.vector.tensor_copy(ob[:], po[sub][:, 0:H])
            pstr = psA.tile([P, P], BF16, name="tr")
            nc.tensor.transpose(pstr[:], ob[:], ident[:])
            nc.vector.tensor_copy(o_t[:, idx, :], pstr[:])

    def outproj_slot(g):
        for tt in range(g * NSLOT, (g + 1) * NSLOT):
            y = ypool.tile([P, D], F32)
            for half in range(2):
                ps = psA.tile([P, QC], F32)
                nc.tensor.matmul(ps[:], lhsT=o_t[:, tt, :],
                                 rhs=woT_sb[:, bass.ts(half, QC)],
                                 start=True, stop=True)
                nc.vector.tensor_scalar_mul(y[:, bass.ts(half, QC)], ps[:],
                                            rec_sb[:, tt : tt + 1])
            nc.sync.dma_start(out[bass.ts(tt, P), :], y[:])

    # Pipelined emission over key 512-chunks: load the chunk's x columns
    # (already D-major; cast-DMA only), project K/V (+V re-transpose to
    # token-major), project Q when its chunk lands, then run each slot's
    # attention + out-projection as soon as its extent is covered.
    for kt in range(NKT):
        for c in range(DC):
            nc.gpsimd.dma_start(xT[:, c, bass.ts(kt, QC)],
                                xbT[bass.ts(c, P), bass.ts(kt, QC)])
        project(wk_sb, k_sb, kt)
        project(wv_sb, vT_sb, kt)
        for j in range(4):  # PE-transpose V to token-major (keeps DMA in copy mode)
            bk = 4 * kt + j
            pstr = psA.tile([P, P], BF16, name="tr")
            nc.tensor.transpose(pstr[:], vT_sb[:, bass.ts(bk, P)], ident[:])
            nc.vector.tensor_copy(v_sb[:, bk, 0:H], pstr[:])
        if kt % 2 == 0:
            project(wq_sb, q_sb, kt, dst_kt=kt // 2)  # QSLOT[g] == 2g == kt
        else:
            g = (kt - 1) // 2
            attention_slot(g)
            outproj_slot(g)


_CACHED_NC = None


def _get_nc():
    global _CACHED_NC
    if _CACHED_NC is None:
        _CACHED_NC = _build_kernel()
    return _CACHED_NC


def _make_core_inputs(x, wqT, wkT, wvT, woT, core):
    b, role = core // 2, core % 2
    sigma = SIGMA[role]
    perm = np.concatenate([np.arange(QC) + c * QC for c in sigma])
    xbT = np.ascontiguousarray(x[perm, b, :].T, dtype=np.float32)

    # tensor_mask thresholds: keep q_local where -q_local < mlen[k_local],
    # i.e. q_true >= k_true  =>  mlen = Q0 - k_true + 1
    mlen = np.zeros((P, NSLOT * 8), np.float32)
    kk = np.arange(P)
    for g in range(NSLOT):
        q0 = sigma[QSLOT[g]] * QC
        for p in range(8):
            sc = sigma[2 * g + p // 4]
            k_true = sc * QC + (p % 4) * P + kk
            mlen[:, g * 8 + p] = q0 - k_true + 1
    return {"xbT": xbT, "wqT": wqT, "wkT": wkT, "wvT": wvT, "woT": woT,
            "mlen": mlen}


def kernel(x, Wq, Wk, Wv, Wo):
    x = np.asarray(x, dtype=np.float32)
    wqT = np.ascontiguousarray(np.asarray(Wq, np.float32).T)
    wkT = np.ascontiguousarray(np.asarray(Wk, np.float32).T)
    wvT = np.ascontiguousarray(np.asarray(Wv, np.float32).T)
    woT = np.ascontiguousarray(np.asarray(Wo, np.float32).T)

    nc = _get_nc()
    in_maps = [_make_core_inputs(x, wqT, wkT, wvT, woT, i) for i in range(8)]
    res = run_bass_kernel_spmd(nc, in_maps, list(range(8))).results

    out = np.empty((S, B, D), np.float32)
    for core in range(8):
        b, role = core // 2, core % 2
        sigma = SIGMA[role]
        co = res[core]["out"]
        for g in range(NSLOT):
            c_g = sigma[QSLOT[g]]
            out[c_g * QC : (c_g + 1) * QC, b, :] = co[g * QC : (g + 1) * QC, :]
    return out
